# revision 48
# baseline (speedup 1.0000x reference)
"""Additive attention (B=16, S=8192, H=256) on 8 TRN2 NeuronCores.

Data-parallel over batch: each core handles 2 examples; weights replicated.
Per example on device:
  pre[o, s]  = WkT-chunks.T @ keyT-chunks            (f32r matmuls, PSUM accum)
  hidT[o, s] = tanh(pre + c[o])        c = q @ Wq.T + b   (folded on host)
  scores[s]  = v . hidT[:, s]                        (f32r matmul, m=1)
  attn       = softmax(scores) over all S            (column layout [128, 64])
  ctx[h]     = sum_s attn[s] * value[s, h]           (bf16 matmuls)
Outputs: context [B,1,H] f32 and attn_weights [B,S] f32.
"""

import os
import sys

sys.path.insert(0, "/opt/trn_rl_repo")

import numpy as np
import ml_dtypes

B, S, H = 16, 8192, 256
NCORES = 8
BPC = B // NCORES          # examples per core = 2
NSB = S // 512             # 16 sub-blocks of 512
NCH = S // 128             # 64 chunks of 128
KBLK = 4096                # key DMA block (free dim)
NKB = S // KBLK            # key DMA blocks per (example, h-chunk)

LAST = {"exec_time_ns": None, "results": None}

_CACHE = {}


def _ensure_ntff_hook():
    """Install the axon NTFF profile hook if the image's antenv lacks it.

    Mirrors trn_agent_boot.trn_boot's ctypes hook; only used when tracing
    is requested (BASS_TRACE=1). Harmless no-op otherwise.
    """
    import types
    import contextlib
    import ctypes

    try:
        from antenv.axon_hooks import get_axon_ntff_profile_hook  # noqa: F401
        return
    except ImportError:
        pass

    mod = types.ModuleType("antenv.axon_hooks")
    _state = {"hook": None}
    mod.set_axon_ntff_profile_hook = lambda h: _state.__setitem__("hook", h)
    mod.get_axon_ntff_profile_hook = lambda: _state["hook"]
    sys.modules["antenv.axon_hooks"] = mod

    so_path = "/opt/axon/libaxon_pjrt.so"
    if not os.path.exists(so_path):
        return
    lib = ctypes.CDLL(so_path)
    if not hasattr(lib, "axon_start_nrt_profile"):
        return
    lib.axon_start_nrt_profile.argtypes = [
        ctypes.POINTER(ctypes.c_int64), ctypes.c_size_t]
    lib.axon_start_nrt_profile.restype = ctypes.c_int64
    lib.axon_stop_nrt_profile.argtypes = [ctypes.c_char_p]
    lib.axon_stop_nrt_profile.restype = ctypes.c_int64

    @contextlib.contextmanager
    def _hook(output_dir, device_ids):
        import jax
        jax.devices()
        if device_ids:
            ids = (ctypes.c_int64 * len(device_ids))(*device_ids)
            rc = lib.axon_start_nrt_profile(ids, len(device_ids))
        else:
            rc = lib.axon_start_nrt_profile(None, 0)
        if rc != 0:
            raise RuntimeError(f"axon_start_nrt_profile rc={rc}")
        try:
            yield
        finally:
            n = lib.axon_stop_nrt_profile(str(output_dir).encode())
            print(f"ntff profile: {n} file(s) written to {output_dir}",
                  file=sys.stderr)

    mod.set_axon_ntff_profile_hook(_hook)


def _build_nc():
    import concourse.bass as bass
    import concourse.mybir as mybir
    from concourse import tile

    f32 = mybir.dt.float32
    f32r = mybir.dt.float32r
    bf16 = mybir.dt.bfloat16
    Tanh = mybir.ActivationFunctionType.Tanh
    Exp = mybir.ActivationFunctionType.Exp
    AX = mybir.AxisListType.X
    PE = mybir.EngineType.PE
    ACT = mybir.EngineType.Activation
    POOL = mybir.EngineType.Pool

    nc = bass.Bass()

    def dep_nop(engine, aps):
        """NOP on `engine` that reads `aps`: absorbs cross-queue waits so the
        following fused-load matmuls carry at most one sync wait each."""
        eng = nc.engines[engine]
        inst = mybir.InstNoOp(name=nc.get_next_instruction_name(),
                              text_hint="dep")
        inst.engine = engine
        inst.ins = [eng.lower_ap(ap) for ap in aps]
        nc.add_instruction(inst)

    keyT = nc.declare_dram_parameter("keyT", [BPC, 2, 128, S], bf16, isOutput=False)
    val = nc.declare_dram_parameter("val", [BPC, 128, NCH * H], bf16, isOutput=False)
    wkT = nc.declare_dram_parameter("wkT", [128, 2, H], bf16, isOutput=False)
    vvec = nc.declare_dram_parameter("vvec", [128, 2], f32r, isOutput=False)
    # packed f32 constants: cols 0:4 = per-example tanh bias c, 4:20 =
    # 16x16 identity (rows 0:16), col 20:36 row0 = ones, col 36 rows 0:16
    # = exp shift (-20)
    cst = nc.declare_dram_parameter("cst", [128, 40], f32, isOutput=False)
    attn_out = nc.declare_dram_parameter("attn_out", [BPC, NSB, 512], f32, isOutput=True)
    ctx_out = nc.declare_dram_parameter("ctx_out", [BPC, 1, H], f32, isOutput=True)

    with tile.TileContext(nc) as tc:
        with (
            tc.tile_pool(name="const", bufs=1) as constp,
            tc.tile_pool(name="kt", bufs=2) as ktp,
            tc.tile_pool(name="valp", bufs=2) as valp,
            tc.tile_pool(name="hid", bufs=6) as hidp,
            tc.tile_pool(name="sm", bufs=2) as smp,
            tc.tile_pool(name="psh", bufs=4, space=bass.MemorySpace.PSUM) as pshp,
            tc.tile_pool(name="pssc", bufs=2, space=bass.MemorySpace.PSUM) as psscp,
            tc.tile_pool(name="psT", bufs=1, space=bass.MemorySpace.PSUM) as psTp,
            tc.tile_pool(name="psctx", bufs=1, space=bass.MemorySpace.PSUM) as psctxp,
        ):
            wk_sb = constp.tile([128, 2, H], bf16)
            v_sb = constp.tile([128, 2], f32r)
            cst_sb = constp.tile([128, 40], f32)

            def emit_consts():
                nc.sync.dma_start(wk_sb[:], wkT[:])
                nc.sync.dma_start(v_sb[:], vvec[:])
                nc.sync.dma_start(cst_sb[:], cst[:])

            def init_example(ex):
                st = {"ex": ex, "pending": [], "s_done": 0}
                st["srows"] = smp.tile([1, S], f32, tag="srows", name="srows",
                                       bufs=1)
                st["blocks"] = ([1024, 3072, 4096] if ex == 0
                                else [512, 3584, 2048, 2048])
                return st

            def emit_scores(st, r, h0, h1):
                dep_nop(PE, [h0[:], h1[:]])
                ps_sc = psscp.tile([1, 512], f32, tag="sc", name="ps_sc")
                nc.tensor.matmul(ps_sc[:], v_sb[:, 0:1], h0[:],
                                 start=True, stop=False)
                nc.tensor.matmul(ps_sc[:], v_sb[:, 1:2], h1[:],
                                 start=False, stop=True)
                nc.vector.tensor_copy(st["srows"][0:1, r * 512:(r + 1) * 512],
                                      ps_sc[:])

            def emit_kb_dmas(st, kb):
                ex = st["ex"]
                blk = st["blocks"][kb]
                kt0 = ktp.tile([128, KBLK], bf16, tag="kt0", name="kt0")
                kt1 = ktp.tile([128, KBLK], bf16, tag="kt1", name="kt1")
                ksl = slice(st["s_done"], st["s_done"] + blk)
                nc.sync.dma_start(kt0[:, 0:blk], keyT[ex, 0, :, ksl])
                nc.sync.dma_start(kt1[:, 0:blk], keyT[ex, 1, :, ksl])
                st["kt"] = (kt0, kt1)

            def emit_kb(st, kb, dmas_done=False):
                ex = st["ex"]
                blk = st["blocks"][kb]
                if not dmas_done:
                    emit_kb_dmas(st, kb)
                kt0, kt1 = st["kt"]
                if kb == 0:
                    st["val_t"] = valp.tile([128, NCH * H], bf16, tag="val",
                                            name="val_t")
                else:
                    # value chunk rides the same FIFO ring *behind* this key
                    # block: keys always win the HBM bandwidth race
                    nv = len(st["blocks"]) - 1
                    vsl = slice((kb - 1) * (NCH * H) // nv,
                                kb * (NCH * H) // nv)
                    nc.sync.dma_start(st["val_t"][:, vsl], val[ex][:, vsl])
                dep_nop(PE, [kt0[:]])
                dep_nop(PE, [kt1[:]])

                for sb in range(blk // 512):
                    r = (st["s_done"] // 512) + sb
                    sl = slice(sb * 512, (sb + 1) * 512)
                    ph0 = pshp.tile([128, 512], f32, tag="ph", name="ph0")
                    ph1 = pshp.tile([128, 512], f32, tag="ph", name="ph1")
                    m0 = kt0[:, sl]
                    m1 = kt1[:, sl]
                    nc.tensor.matmul(ph0[:], wk_sb[:, 0, 0:128], m0,
                                     start=True, stop=False)
                    nc.tensor.matmul(ph0[:], wk_sb[:, 1, 0:128], m1,
                                     start=False, stop=True)
                    nc.tensor.matmul(ph1[:], wk_sb[:, 0, 128:256], m0,
                                     start=True, stop=False)
                    nc.tensor.matmul(ph1[:], wk_sb[:, 1, 128:256], m1,
                                     start=False, stop=True)

                    h0 = hidp.tile([128, 512], f32r, tag="h0", name="h0")
                    h1 = hidp.tile([128, 512], f32r, tag="h1", name="h1")
                    nc.scalar.activation(h0[:], ph0[:], Tanh,
                                         bias=cst_sb[:, 2 * ex:2 * ex + 1])
                    nc.scalar.activation(h1[:], ph1[:], Tanh,
                                         bias=cst_sb[:, 2 * ex + 1:2 * ex + 2])

                    st["pending"].append((r, h0, h1))
                    if len(st["pending"]) >= 2:
                        pr, ph0_, ph1_ = st["pending"].pop(0)
                        emit_scores(st, pr, ph0_, ph1_)
                st["s_done"] += blk

            def finish_scores(st):
                for pr, ph0_, ph1_ in st["pending"]:
                    emit_scores(st, pr, ph0_, ph1_)
                st["pending"] = []

            def emit_gather_lo(st):
                # bulk of the [1, 8192] score row into [16, 512] rows 0..13
                # (SWDGE: keeps the SP/ACT HWDGE rings free of this latency);
                # rows 14/15 are still in flight at this point
                s16 = smp.tile([NSB, 512], f32, tag="s16", name="s16")
                # note: a single rearranged-AP gather loses its read
                # dependency in Tile's tracker (races on HW); per-row
                # plain-slice DMAs track correctly
                for r in range(14):
                    nc.gpsimd.dma_start(
                        s16[r:r + 1, :],
                        st["srows"][0:1, r * 512:(r + 1) * 512])
                st["s16"] = s16

            def emit_gather(st, hwdge=False):
                s16 = st["s16"]
                eng = nc.sync if hwdge else nc.gpsimd
                eng.dma_start(s16[14:15, :],
                              st["srows"][0:1, 14 * 512:15 * 512])
                eng.dma_start(s16[15:16, :],
                              st["srows"][0:1, 15 * 512:16 * 512])

            def emit_exp(st):
                s16 = st["s16"]
                e16 = smp.tile([NSB, 512], f32, tag="e16", name="e16")
                s16s = smp.tile([NSB, 1], f32, tag="s16s", name="s16s")
                # constant shift keeps exp in the ACT LUT's accurate range
                # (scores for this distribution are ~[-30, 30]); softmax is
                # shift-invariant so the result is exact
                nc.scalar.activation(e16[:], s16[:], Exp, bias=cst_sb[0:16, 36:37],
                                     accum_out=s16s[:])
                st["e16"] = e16
                st["s16s"] = s16s

            def emit_sums(st):
                ex = st["ex"]
                e16, s16s = st["e16"], st["s16s"]
                dep_nop(PE, [s16s[:]])
                ptC = psTp.tile([1, NSB], f32, tag="small", name="ptC")
                nc.tensor.transpose(ptC[:], s16s[:], cst_sb[0:NSB, 4:20])
                sT = smp.tile([1, NSB], f32, tag="sT", name="sT")
                nc.vector.tensor_copy(sT[:], ptC[:])
                tot = smp.tile([1, 1], f32, tag="tot", name="tot")
                nc.vector.reduce_sum(tot[:], sT[:], axis=AX)
                rtot = smp.tile([1, 1], f32, tag="rtot", name="rtot")
                nc.vector.reciprocal(rtot[:], tot[:])
                st["rtot"] = rtot
                ptD = psTp.tile([NSB, 1], f32, tag="small", name="ptD")
                nc.tensor.matmul(ptD[:], cst_sb[0:1, 20:36], rtot[:],
                                 start=True, stop=True)  # rtot bcast [16,1]
                r16 = smp.tile([NSB, 1], f32, tag="r16", name="r16")
                nc.vector.tensor_copy(r16[:], ptD[:])

                attn_sb = smp.tile([NSB, 512], f32, tag="attnsb",
                                   name="attn_sb")
                nc.vector.tensor_scalar_mul(attn_sb[:], e16[:], r16[:])
                nc.sync.dma_start(attn_out[ex], attn_sb[:])

            def emit_ebfT(st):
                # unnormalized e -> bf16 columns via DMA transpose (xbar);
                # context accumulates on e and is scaled by 1/total at the
                # end, so the transposes don't wait for the softmax total
                e_bf = smp.tile([NSB, 512], bf16, tag="ebf", name="e_bf")
                nc.vector.tensor_copy(e_bf[:], st["e16"][:])
                eTb = smp.tile([128, 4, NSB], bf16, tag="eTb", name="eTb")
                for q in range(4):
                    nc.sync.dma_start(eTb[:, q, :],
                                      e_bf[:, q * 128:(q + 1) * 128],
                                      transpose=True)
                st["eTb"] = eTb

            def emit_ctx(st):
                ex = st["ex"]
                val_t, eTb, rtot = st["val_t"], st["eTb"], st["rtot"]
                dep_nop(PE, [val_t[:]])
                ps_ctx = psctxp.tile([1, H], f32, tag="ctx", name="ps_ctx")
                # grouped by transpose q so each group starts as soon as its
                # xbar transpose lands
                for i, q in enumerate(range(4)):
                    dep_nop(PE, [eTb[:, q, :]])
                    for rr in range(NSB):
                        n = 4 * rr + q
                        nc.tensor.matmul(ps_ctx[:], eTb[:, q, rr:rr + 1],
                                         val_t[:, n * H:(n + 1) * H],
                                         start=(i == 0 and rr == 0),
                                         stop=(i == 3 and rr == NSB - 1))
                ctx_sb = smp.tile([1, H], f32, tag="ctxsb", name="ctx_sb")
                nc.vector.tensor_scalar_mul(ctx_sb[:], ps_ctx[:], rtot[:])
                nc.sync.dma_start(ctx_out[ex], ctx_sb[:])

            # cross-example software pipeline: ex0's softmax/context DMAs
            # and small ops hide under ex1's streaming phase; ex0's context
            # matmuls cover ex1's score-gather latency at the tail
            st0 = init_example(0)
            emit_kb_dmas(st0, 0)
            emit_consts()
            emit_kb(st0, 0, dmas_done=True)
            for kb in range(1, len(st0["blocks"])):
                emit_kb(st0, kb)
            emit_gather_lo(st0)
            st1 = init_example(1)
            emit_kb(st1, 0)
            finish_scores(st0)
            emit_gather(st0)
            emit_kb(st1, 1)
            emit_exp(st0)
            emit_kb(st1, 2)
            emit_kb(st1, 3)
            emit_sums(st0)
            emit_ebfT(st0)
            emit_gather_lo(st1)
            finish_scores(st1)
            emit_gather(st1, hwdge=True)
            emit_ctx(st0)
            emit_exp(st1)
            emit_sums(st1)
            emit_ebfT(st1)
            emit_ctx(st1)

    _split_excess_waits(nc, mybir)
    if not nc.is_finalized():
        nc.finalize()
    return nc


def _split_excess_waits(nc, mybir):
    """Walrus allows only one sync wait on fused-load (f32/f32r) matmuls.

    Move all but one wait from each Matmult/Ldweights onto a NoOp inserted
    just before it in the same block (same engine stream) — semantically
    identical, waits just fire one instruction earlier.
    """
    for blk in nc.m.functions[0].blocks:
        new = []
        for inst in blk.instructions:
            si = inst.sync_info
            if si is not None and len(si.on_wait) > 1:
                waits = list(si.on_wait)
                for w in waits[:-1]:
                    nop = mybir.InstNoOp(name=nc.get_next_instruction_name(),
                                         text_hint="waitsplit", bass_nofuse=True)
                    nop.engine = inst.engine
                    nop.sync_info = mybir.SyncInfo(on_wait=[w], on_update=[])
                    nc.inst_map[nop.name] = nop
                    new.append(nop)
                inst.sync_info = mybir.SyncInfo(on_wait=[waits[-1]],
                                                on_update=list(si.on_update))
            new.append(inst)
        blk.instructions[:] = new


def _prep_inputs(query, key, value, W_attn, b_attn, v):
    """Host-side shard + relayout. Returns in_maps for 8 cores."""
    query = np.asarray(query, np.float32)
    key = np.asarray(key, np.float32)
    value = np.asarray(value, np.float32)
    W_attn = np.asarray(W_attn, np.float32)
    b_attn = np.asarray(b_attn, np.float32)
    v = np.asarray(v, np.float32)

    Wq = W_attn[:, :H]
    c_all = query[:, 0, :] @ Wq.T + b_attn          # [B, H]
    wkT_h = np.ascontiguousarray(
        W_attn[:, H:].T.reshape(2, 128, H).transpose(1, 0, 2)).astype(
            ml_dtypes.bfloat16)                                     # [128, 2, H]
    v_h = np.ascontiguousarray(v.reshape(2, 128).T)                 # [128, 2]

    keyT = np.ascontiguousarray(
        key.transpose(0, 2, 1).reshape(B, 2, 128, S)).astype(
            ml_dtypes.bfloat16)                                     # [B, 2, 128, S]
    val_r = np.ascontiguousarray(
        value.reshape(B, NCH, 128, H).transpose(0, 2, 1, 3)
        .reshape(B, 128, NCH * H)).astype(ml_dtypes.bfloat16)       # [B, 128, NCH*H]
    c_r = np.ascontiguousarray(
        c_all.reshape(B // BPC, BPC, 2, 128).transpose(0, 3, 1, 2))  # [8, 128, BPC, 2]
    cst_h = np.zeros((B // BPC, 128, 40), np.float32)
    cst_h[:, :, 0:4] = c_r.reshape(B // BPC, 128, 4)
    cst_h[:, 0:16, 4:20] = np.eye(16, dtype=np.float32)
    cst_h[:, 0, 20:36] = 1.0
    cst_h[:, 0:16, 36] = -20.0

    in_maps = []
    for c in range(NCORES):
        sl = slice(c * BPC, (c + 1) * BPC)
        in_maps.append({
            "keyT": keyT[sl],
            "val": val_r[sl],
            "wkT": wkT_h,
            "vvec": v_h,
            "cst": cst_h[c],
        })
    return in_maps


def kernel(query, key, value, W_attn, b_attn, v):
    from concourse.bass_utils import run_bass_kernel_spmd

    if "nc" not in _CACHE:
        _CACHE["nc"] = _build_nc()
    nc = _CACHE["nc"]

    in_maps = _prep_inputs(query, key, value, W_attn, b_attn, v)
    trace = bool(os.environ.get("BASS_TRACE"))
    if trace:
        _ensure_ntff_hook()
    res = run_bass_kernel_spmd(nc, in_maps, core_ids=list(range(NCORES)),
                               trace=trace)
    LAST["exec_time_ns"] = res.exec_time_ns
    LAST["results"] = res

    attn = np.concatenate(
        [r["attn_out"].reshape(BPC, S) for r in res.results], axis=0)   # [B, S]
    ctx = np.concatenate(
        [r["ctx_out"].reshape(BPC, 1, H) for r in res.results], axis=0)  # [B, 1, H]
    return ctx.astype(np.float32), attn.astype(np.float32)


# revision 49
# speedup vs baseline: 1.0353x; 1.0353x over previous
"""Additive attention (B=16, S=8192, H=256) on 8 TRN2 NeuronCores.

Data-parallel over batch: each core handles 2 examples; weights replicated.
Per example on device:
  pre[o, s]  = WkT-chunks.T @ keyT-chunks            (f32r matmuls, PSUM accum)
  hidT[o, s] = tanh(pre + c[o])        c = q @ Wq.T + b   (folded on host)
  scores[s]  = v . hidT[:, s]                        (f32r matmul, m=1)
  attn       = softmax(scores) over all S            (column layout [128, 64])
  ctx[h]     = sum_s attn[s] * value[s, h]           (bf16 matmuls)
Outputs: context [B,1,H] f32 and attn_weights [B,S] f32.
"""

import os
import sys

sys.path.insert(0, "/opt/trn_rl_repo")

import numpy as np
import ml_dtypes

B, S, H = 16, 8192, 256
NCORES = 8
BPC = B // NCORES          # examples per core = 2
NSB = S // 512             # 16 sub-blocks of 512
NCH = S // 128             # 64 chunks of 128
KBLK = 4096                # key DMA block (free dim)
NKB = S // KBLK            # key DMA blocks per (example, h-chunk)

LAST = {"exec_time_ns": None, "results": None}

_CACHE = {}


def _ensure_ntff_hook():
    """Install the axon NTFF profile hook if the image's antenv lacks it.

    Mirrors trn_agent_boot.trn_boot's ctypes hook; only used when tracing
    is requested (BASS_TRACE=1). Harmless no-op otherwise.
    """
    import types
    import contextlib
    import ctypes

    try:
        from antenv.axon_hooks import get_axon_ntff_profile_hook  # noqa: F401
        return
    except ImportError:
        pass

    mod = types.ModuleType("antenv.axon_hooks")
    _state = {"hook": None}
    mod.set_axon_ntff_profile_hook = lambda h: _state.__setitem__("hook", h)
    mod.get_axon_ntff_profile_hook = lambda: _state["hook"]
    sys.modules["antenv.axon_hooks"] = mod

    so_path = "/opt/axon/libaxon_pjrt.so"
    if not os.path.exists(so_path):
        return
    lib = ctypes.CDLL(so_path)
    if not hasattr(lib, "axon_start_nrt_profile"):
        return
    lib.axon_start_nrt_profile.argtypes = [
        ctypes.POINTER(ctypes.c_int64), ctypes.c_size_t]
    lib.axon_start_nrt_profile.restype = ctypes.c_int64
    lib.axon_stop_nrt_profile.argtypes = [ctypes.c_char_p]
    lib.axon_stop_nrt_profile.restype = ctypes.c_int64

    @contextlib.contextmanager
    def _hook(output_dir, device_ids):
        import jax
        jax.devices()
        if device_ids:
            ids = (ctypes.c_int64 * len(device_ids))(*device_ids)
            rc = lib.axon_start_nrt_profile(ids, len(device_ids))
        else:
            rc = lib.axon_start_nrt_profile(None, 0)
        if rc != 0:
            raise RuntimeError(f"axon_start_nrt_profile rc={rc}")
        try:
            yield
        finally:
            n = lib.axon_stop_nrt_profile(str(output_dir).encode())
            print(f"ntff profile: {n} file(s) written to {output_dir}",
                  file=sys.stderr)

    mod.set_axon_ntff_profile_hook(_hook)


def _build_nc():
    import concourse.bass as bass
    import concourse.mybir as mybir
    from concourse import tile

    f32 = mybir.dt.float32
    f32r = mybir.dt.float32r
    bf16 = mybir.dt.bfloat16
    Tanh = mybir.ActivationFunctionType.Tanh
    Exp = mybir.ActivationFunctionType.Exp
    AX = mybir.AxisListType.X
    PE = mybir.EngineType.PE
    ACT = mybir.EngineType.Activation
    POOL = mybir.EngineType.Pool

    nc = bass.Bass()

    def dep_nop(engine, aps):
        """NOP on `engine` that reads `aps`: absorbs cross-queue waits so the
        following fused-load matmuls carry at most one sync wait each."""
        eng = nc.engines[engine]
        inst = mybir.InstNoOp(name=nc.get_next_instruction_name(),
                              text_hint="dep")
        inst.engine = engine
        inst.ins = [eng.lower_ap(ap) for ap in aps]
        nc.add_instruction(inst)

    keyT = nc.declare_dram_parameter("keyT", [BPC, 2, 128, S], bf16, isOutput=False)
    val = nc.declare_dram_parameter("val", [BPC, 128, NCH * H], bf16, isOutput=False)
    wkT = nc.declare_dram_parameter("wkT", [128, 2, H], bf16, isOutput=False)
    vvec = nc.declare_dram_parameter("vvec", [128, 2], f32r, isOutput=False)
    # packed f32 constants: cols 0:4 = per-example tanh bias c, 4:20 =
    # 16x16 identity (rows 0:16), col 20:36 row0 = ones, col 36 rows 0:16
    # = exp shift (-20)
    cst = nc.declare_dram_parameter("cst", [128, 40], f32, isOutput=False)
    attn_out = nc.declare_dram_parameter("attn_out", [BPC, NSB, 512], f32, isOutput=True)
    ctx_out = nc.declare_dram_parameter("ctx_out", [BPC, 1, H], f32, isOutput=True)

    with tile.TileContext(nc) as tc:
        with (
            tc.tile_pool(name="const", bufs=1) as constp,
            tc.tile_pool(name="kt", bufs=2) as ktp,
            tc.tile_pool(name="valp", bufs=2) as valp,
            tc.tile_pool(name="hid", bufs=6) as hidp,
            tc.tile_pool(name="sm", bufs=2) as smp,
            tc.tile_pool(name="psh", bufs=4, space=bass.MemorySpace.PSUM) as pshp,
            tc.tile_pool(name="pssc", bufs=2, space=bass.MemorySpace.PSUM) as psscp,
            tc.tile_pool(name="psT", bufs=1, space=bass.MemorySpace.PSUM) as psTp,
            tc.tile_pool(name="psctx", bufs=1, space=bass.MemorySpace.PSUM) as psctxp,
        ):
            wk_sb = constp.tile([128, 2, H], bf16)
            v_sb = constp.tile([128, 2], f32r)
            cst_sb = constp.tile([128, 40], f32)

            def emit_consts():
                nc.sync.dma_start(wk_sb[:], wkT[:])
                nc.sync.dma_start(v_sb[:], vvec[:])
                nc.sync.dma_start(cst_sb[:], cst[:])

            def init_example(ex):
                st = {"ex": ex, "pending": [], "s_done": 0}
                st["srows"] = smp.tile([1, S], f32, tag="srows", name="srows",
                                       bufs=1)
                st["blocks"] = ([1024, 3072, 2048, 2048] if ex == 0
                                else [512, 3584, 2048, 2048])
                return st

            def emit_scores(st, r, h0, h1):
                dep_nop(PE, [h0[:], h1[:]])
                ps_sc = psscp.tile([1, 512], f32, tag="sc", name="ps_sc")
                nc.tensor.matmul(ps_sc[:], v_sb[:, 0:1], h0[:],
                                 start=True, stop=False)
                nc.tensor.matmul(ps_sc[:], v_sb[:, 1:2], h1[:],
                                 start=False, stop=True)
                nc.vector.tensor_copy(st["srows"][0:1, r * 512:(r + 1) * 512],
                                      ps_sc[:])

            def emit_kb_dmas(st, kb):
                ex = st["ex"]
                blk = st["blocks"][kb]
                kt0 = ktp.tile([128, KBLK], bf16, tag="kt0", name="kt0")
                kt1 = ktp.tile([128, KBLK], bf16, tag="kt1", name="kt1")
                ksl = slice(st["s_done"], st["s_done"] + blk)
                nc.sync.dma_start(kt0[:, 0:blk], keyT[ex, 0, :, ksl])
                nc.sync.dma_start(kt1[:, 0:blk], keyT[ex, 1, :, ksl])
                st["kt"] = (kt0, kt1)

            def emit_kb(st, kb, dmas_done=False):
                ex = st["ex"]
                blk = st["blocks"][kb]
                if not dmas_done:
                    emit_kb_dmas(st, kb)
                kt0, kt1 = st["kt"]
                if kb == 0:
                    st["val_t"] = valp.tile([128, NCH * H], bf16, tag="val",
                                            name="val_t")
                else:
                    # value chunk rides the same FIFO ring *behind* this key
                    # block: keys always win the HBM bandwidth race
                    nv = len(st["blocks"]) - 1
                    vsl = slice((kb - 1) * (NCH * H) // nv,
                                kb * (NCH * H) // nv)
                    nc.sync.dma_start(st["val_t"][:, vsl], val[ex][:, vsl])
                dep_nop(PE, [kt0[:]])
                dep_nop(PE, [kt1[:]])

                for sb in range(blk // 512):
                    r = (st["s_done"] // 512) + sb
                    sl = slice(sb * 512, (sb + 1) * 512)
                    ph0 = pshp.tile([128, 512], f32, tag="ph", name="ph0")
                    ph1 = pshp.tile([128, 512], f32, tag="ph", name="ph1")
                    m0 = kt0[:, sl]
                    m1 = kt1[:, sl]
                    nc.tensor.matmul(ph0[:], wk_sb[:, 0, 0:128], m0,
                                     start=True, stop=False)
                    nc.tensor.matmul(ph0[:], wk_sb[:, 1, 0:128], m1,
                                     start=False, stop=True)
                    nc.tensor.matmul(ph1[:], wk_sb[:, 0, 128:256], m0,
                                     start=True, stop=False)
                    nc.tensor.matmul(ph1[:], wk_sb[:, 1, 128:256], m1,
                                     start=False, stop=True)

                    h0 = hidp.tile([128, 512], f32r, tag="h0", name="h0")
                    h1 = hidp.tile([128, 512], f32r, tag="h1", name="h1")
                    nc.scalar.activation(h0[:], ph0[:], Tanh,
                                         bias=cst_sb[:, 2 * ex:2 * ex + 1])
                    nc.scalar.activation(h1[:], ph1[:], Tanh,
                                         bias=cst_sb[:, 2 * ex + 1:2 * ex + 2])

                    st["pending"].append((r, h0, h1))
                    if len(st["pending"]) >= 2:
                        pr, ph0_, ph1_ = st["pending"].pop(0)
                        emit_scores(st, pr, ph0_, ph1_)
                st["s_done"] += blk

            def finish_scores(st):
                for pr, ph0_, ph1_ in st["pending"]:
                    emit_scores(st, pr, ph0_, ph1_)
                st["pending"] = []

            def emit_gather_lo(st):
                # bulk of the [1, 8192] score row into [16, 512] rows 0..13
                # (SWDGE: keeps the SP/ACT HWDGE rings free of this latency);
                # rows 14/15 are still in flight at this point
                s16 = smp.tile([NSB, 512], f32, tag="s16", name="s16")
                # note: a single rearranged-AP gather loses its read
                # dependency in Tile's tracker (races on HW); per-row
                # plain-slice DMAs track correctly
                for r in range(14):
                    nc.gpsimd.dma_start(
                        s16[r:r + 1, :],
                        st["srows"][0:1, r * 512:(r + 1) * 512])
                st["s16"] = s16

            def emit_gather(st, hwdge=False):
                s16 = st["s16"]
                eng = nc.sync if hwdge else nc.gpsimd
                eng.dma_start(s16[14:15, :],
                              st["srows"][0:1, 14 * 512:15 * 512])
                eng.dma_start(s16[15:16, :],
                              st["srows"][0:1, 15 * 512:16 * 512])

            def emit_exp(st):
                s16 = st["s16"]
                e16 = smp.tile([NSB, 512], f32, tag="e16", name="e16")
                s16s = smp.tile([NSB, 1], f32, tag="s16s", name="s16s")
                # constant shift keeps exp in the ACT LUT's accurate range
                # (scores for this distribution are ~[-30, 30]); softmax is
                # shift-invariant so the result is exact
                nc.scalar.activation(e16[:], s16[:], Exp, bias=cst_sb[0:16, 36:37],
                                     accum_out=s16s[:])
                st["e16"] = e16
                st["s16s"] = s16s

            def emit_sums(st):
                ex = st["ex"]
                e16, s16s = st["e16"], st["s16s"]
                dep_nop(PE, [s16s[:]])
                ptC = psTp.tile([1, NSB], f32, tag="small", name="ptC")
                nc.tensor.transpose(ptC[:], s16s[:], cst_sb[0:NSB, 4:20])
                sT = smp.tile([1, NSB], f32, tag="sT", name="sT")
                nc.vector.tensor_copy(sT[:], ptC[:])
                tot = smp.tile([1, 1], f32, tag="tot", name="tot")
                nc.vector.reduce_sum(tot[:], sT[:], axis=AX)
                rtot = smp.tile([1, 1], f32, tag="rtot", name="rtot")
                nc.vector.reciprocal(rtot[:], tot[:])
                st["rtot"] = rtot
                ptD = psTp.tile([NSB, 1], f32, tag="small", name="ptD")
                nc.tensor.matmul(ptD[:], cst_sb[0:1, 20:36], rtot[:],
                                 start=True, stop=True)  # rtot bcast [16,1]
                r16 = smp.tile([NSB, 1], f32, tag="r16", name="r16")
                nc.vector.tensor_copy(r16[:], ptD[:])

                attn_sb = smp.tile([NSB, 512], f32, tag="attnsb",
                                   name="attn_sb")
                nc.vector.tensor_scalar_mul(attn_sb[:], e16[:], r16[:])
                nc.sync.dma_start(attn_out[ex], attn_sb[:])

            def emit_ebfT(st):
                # unnormalized e -> bf16 columns via DMA transpose (xbar);
                # context accumulates on e and is scaled by 1/total at the
                # end, so the transposes don't wait for the softmax total
                e_bf = smp.tile([NSB, 512], bf16, tag="ebf", name="e_bf")
                nc.vector.tensor_copy(e_bf[:], st["e16"][:])
                eTb = smp.tile([128, 4, NSB], bf16, tag="eTb", name="eTb")
                for q in range(4):
                    nc.sync.dma_start(eTb[:, q, :],
                                      e_bf[:, q * 128:(q + 1) * 128],
                                      transpose=True)
                st["eTb"] = eTb

            def emit_ctx(st):
                ex = st["ex"]
                val_t, eTb, rtot = st["val_t"], st["eTb"], st["rtot"]
                dep_nop(PE, [val_t[:]])
                ps_ctx = psctxp.tile([1, H], f32, tag="ctx", name="ps_ctx")
                # grouped by transpose q so each group starts as soon as its
                # xbar transpose lands
                for i, q in enumerate(range(4)):
                    dep_nop(PE, [eTb[:, q, :]])
                    for rr in range(NSB):
                        n = 4 * rr + q
                        nc.tensor.matmul(ps_ctx[:], eTb[:, q, rr:rr + 1],
                                         val_t[:, n * H:(n + 1) * H],
                                         start=(i == 0 and rr == 0),
                                         stop=(i == 3 and rr == NSB - 1))
                ctx_sb = smp.tile([1, H], f32, tag="ctxsb", name="ctx_sb")
                nc.vector.tensor_scalar_mul(ctx_sb[:], ps_ctx[:], rtot[:])
                nc.sync.dma_start(ctx_out[ex], ctx_sb[:])

            # cross-example software pipeline: ex0's softmax/context DMAs
            # and small ops hide under ex1's streaming phase; ex0's context
            # matmuls cover ex1's score-gather latency at the tail
            st0 = init_example(0)
            emit_kb_dmas(st0, 0)
            emit_consts()
            emit_kb(st0, 0, dmas_done=True)
            for kb in range(1, len(st0["blocks"])):
                emit_kb(st0, kb)
            emit_gather_lo(st0)
            st1 = init_example(1)
            emit_kb(st1, 0)
            finish_scores(st0)
            emit_gather(st0)
            emit_kb(st1, 1)
            emit_exp(st0)
            emit_kb(st1, 2)
            emit_kb_dmas(st1, 3)
            emit_sums(st0)
            emit_ebfT(st0)
            emit_kb(st1, 3, dmas_done=True)
            emit_gather_lo(st1)
            finish_scores(st1)
            emit_gather(st1, hwdge=True)
            emit_ctx(st0)
            emit_exp(st1)
            emit_sums(st1)
            emit_ebfT(st1)
            emit_ctx(st1)

    _split_excess_waits(nc, mybir)
    if not nc.is_finalized():
        nc.finalize()
    return nc


def _split_excess_waits(nc, mybir):
    """Walrus allows only one sync wait on fused-load (f32/f32r) matmuls.

    Move all but one wait from each Matmult/Ldweights onto a NoOp inserted
    just before it in the same block (same engine stream) — semantically
    identical, waits just fire one instruction earlier.
    """
    for blk in nc.m.functions[0].blocks:
        new = []
        for inst in blk.instructions:
            si = inst.sync_info
            if si is not None and len(si.on_wait) > 1:
                waits = list(si.on_wait)
                for w in waits[:-1]:
                    nop = mybir.InstNoOp(name=nc.get_next_instruction_name(),
                                         text_hint="waitsplit", bass_nofuse=True)
                    nop.engine = inst.engine
                    nop.sync_info = mybir.SyncInfo(on_wait=[w], on_update=[])
                    nc.inst_map[nop.name] = nop
                    new.append(nop)
                inst.sync_info = mybir.SyncInfo(on_wait=[waits[-1]],
                                                on_update=list(si.on_update))
            new.append(inst)
        blk.instructions[:] = new


def _prep_inputs(query, key, value, W_attn, b_attn, v):
    """Host-side shard + relayout. Returns in_maps for 8 cores."""
    query = np.asarray(query, np.float32)
    key = np.asarray(key, np.float32)
    value = np.asarray(value, np.float32)
    W_attn = np.asarray(W_attn, np.float32)
    b_attn = np.asarray(b_attn, np.float32)
    v = np.asarray(v, np.float32)

    Wq = W_attn[:, :H]
    c_all = query[:, 0, :] @ Wq.T + b_attn          # [B, H]
    wkT_h = np.ascontiguousarray(
        W_attn[:, H:].T.reshape(2, 128, H).transpose(1, 0, 2)).astype(
            ml_dtypes.bfloat16)                                     # [128, 2, H]
    v_h = np.ascontiguousarray(v.reshape(2, 128).T)                 # [128, 2]

    keyT = np.ascontiguousarray(
        key.transpose(0, 2, 1).reshape(B, 2, 128, S)).astype(
            ml_dtypes.bfloat16)                                     # [B, 2, 128, S]
    val_r = np.ascontiguousarray(
        value.reshape(B, NCH, 128, H).transpose(0, 2, 1, 3)
        .reshape(B, 128, NCH * H)).astype(ml_dtypes.bfloat16)       # [B, 128, NCH*H]
    c_r = np.ascontiguousarray(
        c_all.reshape(B // BPC, BPC, 2, 128).transpose(0, 3, 1, 2))  # [8, 128, BPC, 2]
    cst_h = np.zeros((B // BPC, 128, 40), np.float32)
    cst_h[:, :, 0:4] = c_r.reshape(B // BPC, 128, 4)
    cst_h[:, 0:16, 4:20] = np.eye(16, dtype=np.float32)
    cst_h[:, 0, 20:36] = 1.0
    cst_h[:, 0:16, 36] = -20.0

    in_maps = []
    for c in range(NCORES):
        sl = slice(c * BPC, (c + 1) * BPC)
        in_maps.append({
            "keyT": keyT[sl],
            "val": val_r[sl],
            "wkT": wkT_h,
            "vvec": v_h,
            "cst": cst_h[c],
        })
    return in_maps


def kernel(query, key, value, W_attn, b_attn, v):
    from concourse.bass_utils import run_bass_kernel_spmd

    if "nc" not in _CACHE:
        _CACHE["nc"] = _build_nc()
    nc = _CACHE["nc"]

    in_maps = _prep_inputs(query, key, value, W_attn, b_attn, v)
    trace = bool(os.environ.get("BASS_TRACE"))
    if trace:
        _ensure_ntff_hook()
    res = run_bass_kernel_spmd(nc, in_maps, core_ids=list(range(NCORES)),
                               trace=trace)
    LAST["exec_time_ns"] = res.exec_time_ns
    LAST["results"] = res

    attn = np.concatenate(
        [r["attn_out"].reshape(BPC, S) for r in res.results], axis=0)   # [B, S]
    ctx = np.concatenate(
        [r["ctx_out"].reshape(BPC, 1, H) for r in res.results], axis=0)  # [B, 1, H]
    return ctx.astype(np.float32), attn.astype(np.float32)


# revision 51
# speedup vs baseline: 1.0473x; 1.0116x over previous
"""Additive attention (B=16, S=8192, H=256) on 8 TRN2 NeuronCores.

Data-parallel over batch: each core handles 2 examples; weights replicated.
Host prep: fold q@Wq.T+b into a per-example bias c, pre-transpose key to
[h, s] layout (bf16), chunk-relayout value (bf16).

Per example on device (cross-example software-pipelined):
  pre[o, s]  = WkT-chunks.T @ keyT-chunks        (bf16 matmuls, f32 PSUM)
  hidT[o, s] = tanh(pre + c[o])                  (ACT, f32r out)
  scores[s]  = v . hidT[:, s]                    (f32r matmul, m=1)
  softmax    = exp(s - 20) rows [16, 512]        (shift-invariant, LUT-safe)
  ctx[h]     = sum_s e[s]*value[s, h] / sum e    (bf16 matmuls; attn columns
                                                  made by bf16 xbar DMA
                                                  transposes)
Outputs: context [B,1,H] f32 and attn_weights [B,S] f32.
HW exec ~98 us on silicon (memory roofline ~47 us at bf16 key+value).
"""

import os
import sys

sys.path.insert(0, "/opt/trn_rl_repo")

import numpy as np
import ml_dtypes

B, S, H = 16, 8192, 256
NCORES = 8
BPC = B // NCORES          # examples per core = 2
NSB = S // 512             # 16 sub-blocks of 512
NCH = S // 128             # 64 chunks of 128
KBLK = 4096                # key DMA block (free dim)
NKB = S // KBLK            # key DMA blocks per (example, h-chunk)

LAST = {"exec_time_ns": None, "results": None}

_CACHE = {}


def _ensure_ntff_hook():
    """Install the axon NTFF profile hook if the image's antenv lacks it.

    Mirrors trn_agent_boot.trn_boot's ctypes hook; only used when tracing
    is requested (BASS_TRACE=1). Harmless no-op otherwise.
    """
    import types
    import contextlib
    import ctypes

    try:
        from antenv.axon_hooks import get_axon_ntff_profile_hook  # noqa: F401
        return
    except ImportError:
        pass

    mod = types.ModuleType("antenv.axon_hooks")
    _state = {"hook": None}
    mod.set_axon_ntff_profile_hook = lambda h: _state.__setitem__("hook", h)
    mod.get_axon_ntff_profile_hook = lambda: _state["hook"]
    sys.modules["antenv.axon_hooks"] = mod

    so_path = "/opt/axon/libaxon_pjrt.so"
    if not os.path.exists(so_path):
        return
    lib = ctypes.CDLL(so_path)
    if not hasattr(lib, "axon_start_nrt_profile"):
        return
    lib.axon_start_nrt_profile.argtypes = [
        ctypes.POINTER(ctypes.c_int64), ctypes.c_size_t]
    lib.axon_start_nrt_profile.restype = ctypes.c_int64
    lib.axon_stop_nrt_profile.argtypes = [ctypes.c_char_p]
    lib.axon_stop_nrt_profile.restype = ctypes.c_int64

    @contextlib.contextmanager
    def _hook(output_dir, device_ids):
        import jax
        jax.devices()
        if device_ids:
            ids = (ctypes.c_int64 * len(device_ids))(*device_ids)
            rc = lib.axon_start_nrt_profile(ids, len(device_ids))
        else:
            rc = lib.axon_start_nrt_profile(None, 0)
        if rc != 0:
            raise RuntimeError(f"axon_start_nrt_profile rc={rc}")
        try:
            yield
        finally:
            n = lib.axon_stop_nrt_profile(str(output_dir).encode())
            print(f"ntff profile: {n} file(s) written to {output_dir}",
                  file=sys.stderr)

    mod.set_axon_ntff_profile_hook(_hook)


def _build_nc():
    import concourse.bass as bass
    import concourse.mybir as mybir
    from concourse import tile

    f32 = mybir.dt.float32
    f32r = mybir.dt.float32r
    bf16 = mybir.dt.bfloat16
    Tanh = mybir.ActivationFunctionType.Tanh
    Exp = mybir.ActivationFunctionType.Exp
    AX = mybir.AxisListType.X
    PE = mybir.EngineType.PE
    ACT = mybir.EngineType.Activation
    POOL = mybir.EngineType.Pool

    nc = bass.Bass()

    def dep_nop(engine, aps):
        """NOP on `engine` that reads `aps`: absorbs cross-queue waits so the
        following fused-load matmuls carry at most one sync wait each."""
        eng = nc.engines[engine]
        inst = mybir.InstNoOp(name=nc.get_next_instruction_name(),
                              text_hint="dep")
        inst.engine = engine
        inst.ins = [eng.lower_ap(ap) for ap in aps]
        nc.add_instruction(inst)

    keyT = nc.declare_dram_parameter("keyT", [BPC, 2, 128, S], bf16, isOutput=False)
    val = nc.declare_dram_parameter("val", [BPC, 128, NCH * H], bf16, isOutput=False)
    wkT = nc.declare_dram_parameter("wkT", [128, 2, H], bf16, isOutput=False)
    vvec = nc.declare_dram_parameter("vvec", [128, 2], f32r, isOutput=False)
    # packed f32 constants: cols 0:4 = per-example tanh bias c, 4:20 =
    # 16x16 identity (rows 0:16), col 20:36 row0 = ones, col 36 rows 0:16
    # = exp shift (-20)
    cst = nc.declare_dram_parameter("cst", [128, 40], f32, isOutput=False)
    attn_out = nc.declare_dram_parameter("attn_out", [BPC, NSB, 512], f32, isOutput=True)
    ctx_out = nc.declare_dram_parameter("ctx_out", [BPC, 1, H], f32, isOutput=True)

    with tile.TileContext(nc) as tc:
        with (
            tc.tile_pool(name="const", bufs=1) as constp,
            tc.tile_pool(name="kt", bufs=2) as ktp,
            tc.tile_pool(name="valp", bufs=2) as valp,
            tc.tile_pool(name="hid", bufs=6) as hidp,
            tc.tile_pool(name="sm", bufs=2) as smp,
            tc.tile_pool(name="psh", bufs=4, space=bass.MemorySpace.PSUM) as pshp,
            tc.tile_pool(name="pssc", bufs=2, space=bass.MemorySpace.PSUM) as psscp,
            tc.tile_pool(name="psT", bufs=1, space=bass.MemorySpace.PSUM) as psTp,
            tc.tile_pool(name="psctx", bufs=1, space=bass.MemorySpace.PSUM) as psctxp,
        ):
            wk_sb = constp.tile([128, 2, H], bf16)
            v_sb = constp.tile([128, 2], f32r)
            cst_sb = constp.tile([128, 40], f32)

            def emit_consts():
                nc.sync.dma_start(wk_sb[:], wkT[:])
                nc.sync.dma_start(v_sb[:], vvec[:])
                nc.sync.dma_start(cst_sb[:], cst[:])

            def init_example(ex):
                st = {"ex": ex, "pending": [], "s_done": 0}
                st["srows"] = smp.tile([1, S], f32, tag="srows", name="srows",
                                       bufs=1)
                st["blocks"] = ([1024, 3072, 2048, 2048] if ex == 0
                                else [512, 3584, 2048, 2048])
                return st

            def emit_scores(st, r, h0, h1):
                dep_nop(PE, [h0[:], h1[:]])
                ps_sc = psscp.tile([1, 512], f32, tag="sc", name="ps_sc")
                nc.tensor.matmul(ps_sc[:], v_sb[:, 0:1], h0[:],
                                 start=True, stop=False)
                nc.tensor.matmul(ps_sc[:], v_sb[:, 1:2], h1[:],
                                 start=False, stop=True)
                nc.vector.tensor_copy(st["srows"][0:1, r * 512:(r + 1) * 512],
                                      ps_sc[:])

            def emit_kb_dmas(st, kb):
                ex = st["ex"]
                blk = st["blocks"][kb]
                kt0 = ktp.tile([128, KBLK], bf16, tag="kt0", name="kt0")
                kt1 = ktp.tile([128, KBLK], bf16, tag="kt1", name="kt1")
                ksl = slice(st["s_done"], st["s_done"] + blk)
                nc.sync.dma_start(kt0[:, 0:blk], keyT[ex, 0, :, ksl])
                nc.sync.dma_start(kt1[:, 0:blk], keyT[ex, 1, :, ksl])
                st["kt"] = (kt0, kt1)

            def emit_kb(st, kb, dmas_done=False):
                ex = st["ex"]
                blk = st["blocks"][kb]
                if not dmas_done:
                    emit_kb_dmas(st, kb)
                kt0, kt1 = st["kt"]
                if kb == 0:
                    st["val_t"] = valp.tile([128, NCH * H], bf16, tag="val",
                                            name="val_t")
                else:
                    # value chunk rides the same FIFO ring *behind* this key
                    # block: keys always win the HBM bandwidth race
                    nv = len(st["blocks"]) - 1
                    vsl = slice((kb - 1) * (NCH * H) // nv,
                                kb * (NCH * H) // nv)
                    nc.sync.dma_start(st["val_t"][:, vsl], val[ex][:, vsl])
                dep_nop(PE, [kt0[:]])
                dep_nop(PE, [kt1[:]])

                for sb in range(blk // 512):
                    r = (st["s_done"] // 512) + sb
                    sl = slice(sb * 512, (sb + 1) * 512)
                    ph0 = pshp.tile([128, 512], f32, tag="ph", name="ph0")
                    ph1 = pshp.tile([128, 512], f32, tag="ph", name="ph1")
                    m0 = kt0[:, sl]
                    m1 = kt1[:, sl]
                    nc.tensor.matmul(ph0[:], wk_sb[:, 0, 0:128], m0,
                                     start=True, stop=False)
                    nc.tensor.matmul(ph0[:], wk_sb[:, 1, 0:128], m1,
                                     start=False, stop=True)
                    nc.tensor.matmul(ph1[:], wk_sb[:, 0, 128:256], m0,
                                     start=True, stop=False)
                    nc.tensor.matmul(ph1[:], wk_sb[:, 1, 128:256], m1,
                                     start=False, stop=True)

                    h0 = hidp.tile([128, 512], f32r, tag="h0", name="h0")
                    h1 = hidp.tile([128, 512], f32r, tag="h1", name="h1")
                    nc.scalar.activation(h0[:], ph0[:], Tanh,
                                         bias=cst_sb[:, 2 * ex:2 * ex + 1])
                    nc.scalar.activation(h1[:], ph1[:], Tanh,
                                         bias=cst_sb[:, 2 * ex + 1:2 * ex + 2])

                    st["pending"].append((r, h0, h1))
                    if len(st["pending"]) >= 2:
                        pr, ph0_, ph1_ = st["pending"].pop(0)
                        emit_scores(st, pr, ph0_, ph1_)
                st["s_done"] += blk

            def finish_scores(st):
                for pr, ph0_, ph1_ in st["pending"]:
                    emit_scores(st, pr, ph0_, ph1_)
                st["pending"] = []

            def emit_gather_lo(st):
                # bulk of the [1, 8192] score row into [16, 512] rows 0..13
                # (SWDGE: keeps the SP/ACT HWDGE rings free of this latency);
                # rows 14/15 are still in flight at this point
                s16 = smp.tile([NSB, 512], f32, tag="s16", name="s16")
                # note: a single rearranged-AP gather loses its read
                # dependency in Tile's tracker (races on HW); per-row
                # plain-slice DMAs track correctly
                for r in range(14):
                    nc.gpsimd.dma_start(
                        s16[r:r + 1, :],
                        st["srows"][0:1, r * 512:(r + 1) * 512])
                st["s16"] = s16

            def emit_gather(st, hwdge=False):
                s16 = st["s16"]
                eng = nc.sync if hwdge else nc.gpsimd
                eng.dma_start(s16[14:15, :],
                              st["srows"][0:1, 14 * 512:15 * 512])
                eng.dma_start(s16[15:16, :],
                              st["srows"][0:1, 15 * 512:16 * 512])

            def emit_exp(st):
                s16 = st["s16"]
                e16 = smp.tile([NSB, 512], f32, tag="e16", name="e16")
                s16s = smp.tile([NSB, 1], f32, tag="s16s", name="s16s")
                # constant shift keeps exp in the ACT LUT's accurate range
                # (scores for this distribution are ~[-30, 30]); softmax is
                # shift-invariant so the result is exact
                nc.scalar.activation(e16[:], s16[:], Exp, bias=cst_sb[0:16, 36:37],
                                     accum_out=s16s[:])
                st["e16"] = e16
                st["s16s"] = s16s

            def emit_sums(st):
                ex = st["ex"]
                e16, s16s = st["e16"], st["s16s"]
                dep_nop(PE, [s16s[:]])
                ptC = psTp.tile([1, NSB], f32, tag="small", name="ptC")
                nc.tensor.transpose(ptC[:], s16s[:], cst_sb[0:NSB, 4:20])
                sT = smp.tile([1, NSB], f32, tag="sT", name="sT")
                nc.vector.tensor_copy(sT[:], ptC[:])
                tot = smp.tile([1, 1], f32, tag="tot", name="tot")
                nc.vector.reduce_sum(tot[:], sT[:], axis=AX)
                rtot = smp.tile([1, 1], f32, tag="rtot", name="rtot")
                nc.vector.reciprocal(rtot[:], tot[:])
                st["rtot"] = rtot
                ptD = psTp.tile([NSB, 1], f32, tag="small", name="ptD")
                nc.tensor.matmul(ptD[:], cst_sb[0:1, 20:36], rtot[:],
                                 start=True, stop=True)  # rtot bcast [16,1]
                r16 = smp.tile([NSB, 1], f32, tag="r16", name="r16")
                nc.vector.tensor_copy(r16[:], ptD[:])

                attn_sb = smp.tile([NSB, 512], f32, tag="attnsb",
                                   name="attn_sb")
                nc.vector.tensor_scalar_mul(attn_sb[:], e16[:], r16[:])
                nc.sync.dma_start(attn_out[ex], attn_sb[:])

            def emit_ebfT(st):
                # unnormalized e -> bf16 columns via DMA transpose (xbar);
                # context accumulates on e and is scaled by 1/total at the
                # end, so the transposes don't wait for the softmax total
                e_bf = smp.tile([NSB, 512], bf16, tag="ebf", name="e_bf")
                nc.vector.tensor_copy(e_bf[:], st["e16"][:])
                eTb = smp.tile([128, 4, NSB], bf16, tag="eTb", name="eTb")
                for q in range(4):
                    nc.sync.dma_start(eTb[:, q, :],
                                      e_bf[:, q * 128:(q + 1) * 128],
                                      transpose=True)
                st["eTb"] = eTb

            def emit_ctx(st):
                ex = st["ex"]
                val_t, eTb, rtot = st["val_t"], st["eTb"], st["rtot"]
                dep_nop(PE, [val_t[:]])
                ps_ctx = psctxp.tile([1, H], f32, tag="ctx", name="ps_ctx")
                # grouped by transpose q so each group starts as soon as its
                # xbar transpose lands
                for i, q in enumerate(range(4)):
                    dep_nop(PE, [eTb[:, q, :]])
                    for rr in range(NSB):
                        n = 4 * rr + q
                        nc.tensor.matmul(ps_ctx[:], eTb[:, q, rr:rr + 1],
                                         val_t[:, n * H:(n + 1) * H],
                                         start=(i == 0 and rr == 0),
                                         stop=(i == 3 and rr == NSB - 1))
                ctx_sb = smp.tile([1, H], f32, tag="ctxsb", name="ctx_sb")
                nc.vector.tensor_scalar_mul(ctx_sb[:], ps_ctx[:], rtot[:])
                nc.sync.dma_start(ctx_out[ex], ctx_sb[:])

            # cross-example software pipeline: ex0's softmax/context DMAs
            # and small ops hide under ex1's streaming phase; ex0's context
            # matmuls cover ex1's score-gather latency at the tail
            st0 = init_example(0)
            emit_kb_dmas(st0, 0)
            emit_consts()
            emit_kb(st0, 0, dmas_done=True)
            for kb in range(1, len(st0["blocks"])):
                emit_kb(st0, kb)
            emit_gather_lo(st0)
            st1 = init_example(1)
            emit_kb(st1, 0)
            finish_scores(st0)
            emit_gather(st0)
            emit_kb(st1, 1)
            emit_exp(st0)
            emit_kb(st1, 2)
            emit_kb_dmas(st1, 3)
            emit_ebfT(st0)
            emit_kb(st1, 3, dmas_done=True)
            emit_sums(st0)
            emit_gather_lo(st1)
            finish_scores(st1)
            emit_gather(st1, hwdge=True)
            emit_ctx(st0)
            emit_exp(st1)
            emit_sums(st1)
            emit_ebfT(st1)
            emit_ctx(st1)

    _split_excess_waits(nc, mybir)
    if not nc.is_finalized():
        nc.finalize()
    return nc


def _split_excess_waits(nc, mybir):
    """Walrus allows only one sync wait on fused-load (f32/f32r) matmuls.

    Move all but one wait from each Matmult/Ldweights onto a NoOp inserted
    just before it in the same block (same engine stream) — semantically
    identical, waits just fire one instruction earlier.
    """
    for blk in nc.m.functions[0].blocks:
        new = []
        for inst in blk.instructions:
            si = inst.sync_info
            if si is not None and len(si.on_wait) > 1:
                waits = list(si.on_wait)
                for w in waits[:-1]:
                    nop = mybir.InstNoOp(name=nc.get_next_instruction_name(),
                                         text_hint="waitsplit", bass_nofuse=True)
                    nop.engine = inst.engine
                    nop.sync_info = mybir.SyncInfo(on_wait=[w], on_update=[])
                    nc.inst_map[nop.name] = nop
                    new.append(nop)
                inst.sync_info = mybir.SyncInfo(on_wait=[waits[-1]],
                                                on_update=list(si.on_update))
            new.append(inst)
        blk.instructions[:] = new


def _prep_inputs(query, key, value, W_attn, b_attn, v):
    """Host-side shard + relayout. Returns in_maps for 8 cores."""
    query = np.asarray(query, np.float32)
    key = np.asarray(key, np.float32)
    value = np.asarray(value, np.float32)
    W_attn = np.asarray(W_attn, np.float32)
    b_attn = np.asarray(b_attn, np.float32)
    v = np.asarray(v, np.float32)

    Wq = W_attn[:, :H]
    c_all = query[:, 0, :] @ Wq.T + b_attn          # [B, H]
    wkT_h = np.ascontiguousarray(
        W_attn[:, H:].T.reshape(2, 128, H).transpose(1, 0, 2)).astype(
            ml_dtypes.bfloat16)                                     # [128, 2, H]
    v_h = np.ascontiguousarray(v.reshape(2, 128).T)                 # [128, 2]

    keyT = np.ascontiguousarray(
        key.transpose(0, 2, 1).reshape(B, 2, 128, S)).astype(
            ml_dtypes.bfloat16)                                     # [B, 2, 128, S]
    val_r = np.ascontiguousarray(
        value.reshape(B, NCH, 128, H).transpose(0, 2, 1, 3)
        .reshape(B, 128, NCH * H)).astype(ml_dtypes.bfloat16)       # [B, 128, NCH*H]
    c_r = np.ascontiguousarray(
        c_all.reshape(B // BPC, BPC, 2, 128).transpose(0, 3, 1, 2))  # [8, 128, BPC, 2]
    cst_h = np.zeros((B // BPC, 128, 40), np.float32)
    cst_h[:, :, 0:4] = c_r.reshape(B // BPC, 128, 4)
    cst_h[:, 0:16, 4:20] = np.eye(16, dtype=np.float32)
    cst_h[:, 0, 20:36] = 1.0
    cst_h[:, 0:16, 36] = -20.0

    in_maps = []
    for c in range(NCORES):
        sl = slice(c * BPC, (c + 1) * BPC)
        in_maps.append({
            "keyT": keyT[sl],
            "val": val_r[sl],
            "wkT": wkT_h,
            "vvec": v_h,
            "cst": cst_h[c],
        })
    return in_maps


def kernel(query, key, value, W_attn, b_attn, v):
    from concourse.bass_utils import run_bass_kernel_spmd

    if "nc" not in _CACHE:
        _CACHE["nc"] = _build_nc()
    nc = _CACHE["nc"]

    in_maps = _prep_inputs(query, key, value, W_attn, b_attn, v)
    trace = bool(os.environ.get("BASS_TRACE"))
    if trace:
        _ensure_ntff_hook()
    res = run_bass_kernel_spmd(nc, in_maps, core_ids=list(range(NCORES)),
                               trace=trace)
    LAST["exec_time_ns"] = res.exec_time_ns
    LAST["results"] = res

    attn = np.concatenate(
        [r["attn_out"].reshape(BPC, S) for r in res.results], axis=0)   # [B, S]
    ctx = np.concatenate(
        [r["ctx_out"].reshape(BPC, 1, H) for r in res.results], axis=0)  # [B, 1, H]
    return ctx.astype(np.float32), attn.astype(np.float32)
